# revision 4
# baseline (speedup 1.0000x reference)
"""GRU unit kernel for Trainium2, data-parallel over 8 NeuronCores.

Computation (per batch row):
    r  = sigmoid(x @ W_i2r + b_i2r + h @ W_h2r)
    z  = sigmoid(x @ W_i2z + b_i2z + h @ W_h2z)
    h1 = tanh   (x @ W_i2h + b_i2h + r * (h @ W_h2h))
    out = (1 - z) * h1 + z * h

Sharding: batch (16384) split 8 ways; weights replicated.

Host-side prep: x/h transposed to [K, B_local] and cast to bf16 (so the
stationary matmul operand loads directly, no on-chip transposes), the three
x-side / h-side weight matrices concatenated to [1024, 3072] bf16, biases
concatenated and pre-broadcast to [128, 3072] f32.

Device kernel per core (B_local=2048 rows = 16 m-tiles of 128):
  - weights + xT/hT resident in SBUF (bf16), h (f32) and out streamed.
  - per m-tile: 96 bf16 matmuls of N=512 into 8 PSUM banks
    (pre_r, pre_z, x@W_i2h, h@W_h2h, each split in two 512-halves),
    fp32 accumulation; epilogue on DVE (bias adds, blend) + ACT
    (sigmoid/tanh); result DMA'd out in fp32.
"""

import numpy as np
import ml_dtypes
from contextlib import ExitStack

import concourse.bass as bass
import concourse.tile as tile
from concourse import bacc, mybir

N_CORES = 8
B, I, H = 16384, 1024, 1024
BL = B // N_CORES           # 2048 batch rows per core
MT = BL // 128              # 16 m-tiles
KO = I // 128               # 8 k-tiles of 128
F32 = mybir.dt.float32
BF16 = mybir.dt.bfloat16
BF16_NP = ml_dtypes.bfloat16


def build_nc():
    nc = bacc.Bacc("TRN2", target_bir_lowering=False, debug=False,
                   num_devices=N_CORES)
    AF = mybir.ActivationFunctionType

    xt = nc.dram_tensor("xt", [I, BL], BF16, kind="ExternalInput").ap()
    ht = nc.dram_tensor("ht", [H, BL], BF16, kind="ExternalInput").ap()
    h32 = nc.dram_tensor("h32", [BL, H], F32, kind="ExternalInput").ap()
    wx = nc.dram_tensor("wx", [I, 3 * H], BF16, kind="ExternalInput").ap()
    wh = nc.dram_tensor("wh", [H, 3 * H], BF16, kind="ExternalInput").ap()
    bias = nc.dram_tensor("bias", [128, 3 * H], F32, kind="ExternalInput").ap()
    out = nc.dram_tensor("out", [BL, H], F32, kind="ExternalOutput").ap()

    with tile.TileContext(nc) as tc, ExitStack() as ctx:
        wpool = ctx.enter_context(tc.tile_pool(name="w", bufs=1))
        apool = ctx.enter_context(tc.tile_pool(name="a", bufs=1))
        hpool = ctx.enter_context(tc.tile_pool(name="h", bufs=2))
        epool = ctx.enter_context(tc.tile_pool(name="e", bufs=3))
        psum = ctx.enter_context(tc.tile_pool(name="ps", bufs=1, space="PSUM"))

        wx_sb = wpool.tile([128, KO, 3 * H], BF16, tag="wx")
        wh_sb = wpool.tile([128, KO, 3 * H], BF16, tag="wh")
        bias_sb = wpool.tile([128, 3 * H], F32, tag="bias")
        xt_sb = apool.tile([128, KO, BL], BF16, tag="xt")
        ht_sb = apool.tile([128, KO, BL], BF16, tag="ht")

        xt_r = xt.rearrange("(ko ki) b -> ki ko b", ki=128)
        ht_r = ht.rearrange("(ko ki) b -> ki ko b", ki=128)
        wx_r = wx.rearrange("(ko ki) n -> ki ko n", ki=128)
        wh_r = wh.rearrange("(ko ki) n -> ki ko n", ki=128)

        # Load order sets DMA FIFO order on the sync ring: first m-tile needs
        # xt chunk 0 + all of wx (x-side), then ht chunk 0 + wh (h-side).
        CH = 4
        cw = BL // CH
        nc.sync.dma_start(xt_sb[:, :, 0:cw], xt_r[:, :, 0:cw])
        nc.sync.dma_start(wx_sb[:], wx_r)
        nc.sync.dma_start(ht_sb[:, :, 0:cw], ht_r[:, :, 0:cw])
        nc.sync.dma_start(wh_sb[:], wh_r)
        nc.sync.dma_start(bias_sb[:], bias)
        for c in range(1, CH):
            nc.sync.dma_start(xt_sb[:, :, c * cw:(c + 1) * cw],
                              xt_r[:, :, c * cw:(c + 1) * cw])
            nc.sync.dma_start(ht_sb[:, :, c * cw:(c + 1) * cw],
                              ht_r[:, :, c * cw:(c + 1) * cw])

        for mt in range(MT):
            ms = slice(mt * 128, (mt + 1) * 128)
            h32_t = hpool.tile([128, H], F32, tag="h32")
            nc.sync.dma_start(h32_t[:], h32[ms, :])

            # 8 PSUM banks: r/z get x-side + h-side accumulated; 'a' is
            # x@W_i2h (+bias later), 'b' is h@W_h2h (scaled by r later).
            ps = {}
            for g in ("r", "z", "a", "b"):
                for nh in range(2):
                    ps[(g, nh)] = psum.tile([128, 512], F32, tag=f"p{g}{nh}",
                                            name=f"p{g}{nh}")

            for ko in range(KO):  # x side: 6 matmuls per stationary tile
                lhsT = xt_sb[:, ko, ms]
                for nh in range(2):
                    o = nh * 512
                    nc.tensor.matmul(ps[("r", nh)], lhsT,
                                     wx_sb[:, ko, o:o + 512],
                                     start=(ko == 0), stop=False)
                    nc.tensor.matmul(ps[("z", nh)], lhsT,
                                     wx_sb[:, ko, H + o:H + o + 512],
                                     start=(ko == 0), stop=False)
                    nc.tensor.matmul(ps[("a", nh)], lhsT,
                                     wx_sb[:, ko, 2 * H + o:2 * H + o + 512],
                                     start=(ko == 0), stop=(ko == KO - 1))
            for ko in range(KO):  # h side
                lhsT = ht_sb[:, ko, ms]
                for nh in range(2):
                    o = nh * 512
                    nc.tensor.matmul(ps[("r", nh)], lhsT,
                                     wh_sb[:, ko, o:o + 512],
                                     start=False, stop=(ko == KO - 1))
                    nc.tensor.matmul(ps[("z", nh)], lhsT,
                                     wh_sb[:, ko, H + o:H + o + 512],
                                     start=False, stop=(ko == KO - 1))
                    nc.tensor.matmul(ps[("b", nh)], lhsT,
                                     wh_sb[:, ko, 2 * H + o:2 * H + o + 512],
                                     start=(ko == 0), stop=(ko == KO - 1))

            for nh in range(2):
                o = nh * 512
                nsl = slice(o, o + 512)
                pr, pz = ps[("r", nh)], ps[("z", nh)]
                pa, pb = ps[("a", nh)], ps[("b", nh)]
                tr = epool.tile([128, 512], F32, tag="tr")
                tz = epool.tile([128, 512], F32, tag="tz")
                ta = epool.tile([128, 512], F32, tag="ta")
                nc.vector.tensor_add(tr[:], pr[:], bias_sb[:, o:o + 512])
                nc.scalar.activation(tr[:], tr[:], AF.Sigmoid)       # r
                nc.vector.tensor_add(tz[:], pz[:], bias_sb[:, H + o:H + o + 512])
                nc.scalar.activation(tz[:], tz[:], AF.Sigmoid)       # z
                nc.vector.tensor_add(ta[:], pa[:],
                                     bias_sb[:, 2 * H + o:2 * H + o + 512])
                nc.vector.tensor_mul(tr[:], tr[:], pb[:])            # r*(hU)
                nc.vector.tensor_add(ta[:], ta[:], tr[:])
                nc.scalar.activation(ta[:], ta[:], AF.Tanh)          # h1
                nc.vector.tensor_sub(tr[:], h32_t[:, nsl], ta[:])    # h-h1
                nc.vector.tensor_mul(tr[:], tz[:], tr[:])            # z*(h-h1)
                nc.vector.tensor_add(tr[:], ta[:], tr[:])            # out
                nc.scalar.dma_start(out[ms, nsl], tr[:])

    nc.compile()
    return nc


def prep_in_maps(inputs):
    """Host-side marshalling: shard batch, transpose+cast activations,
    concat weights/biases. Returns per-core input dicts."""
    g = {k: np.asarray(v) for k, v in inputs.items()}
    x, h = g["inputs"], g["hidden"]
    wx = np.concatenate([g["W_i2r"], g["W_i2z"], g["W_i2h"]], axis=1)
    wh = np.concatenate([g["W_h2r"], g["W_h2z"], g["W_h2h"]], axis=1)
    wx = np.ascontiguousarray(wx).astype(BF16_NP)
    wh = np.ascontiguousarray(wh).astype(BF16_NP)
    b = np.concatenate([g["b_i2r"], g["b_i2z"], g["b_i2h"]]).astype(np.float32)
    bias_b = np.ascontiguousarray(np.broadcast_to(b, (128, 3 * H)))
    xt_all = np.ascontiguousarray(x.T).astype(BF16_NP)
    ht_all = np.ascontiguousarray(h.T).astype(BF16_NP)
    in_maps = []
    for c in range(N_CORES):
        sl = slice(c * BL, (c + 1) * BL)
        in_maps.append({
            "xt": np.ascontiguousarray(xt_all[:, sl]),
            "ht": np.ascontiguousarray(ht_all[:, sl]),
            "h32": np.ascontiguousarray(h[sl].astype(np.float32)),
            "wx": wx,
            "wh": wh,
            "bias": bias_b,
        })
    return in_maps


_RUNNER = None


def get_runner():
    """Build the bass module once and wrap it in a jitted 8-way shard_map,
    mirroring concourse.bass2jax.run_bass_via_pjrt but reusable across calls
    (so repeated executions don't re-trace/re-compile)."""
    global _RUNNER
    if _RUNNER is not None:
        return _RUNNER
    import jax
    from jax.sharding import Mesh, PartitionSpec
    from jax.experimental.shard_map import shard_map
    from concourse.bass2jax import (_bass_exec_p, install_neuronx_cc_hook,
                                    partition_id_tensor)

    nc = build_nc()
    install_neuronx_cc_hook()

    partition_name = (nc.partition_id_tensor.name
                      if nc.partition_id_tensor else None)
    in_names, out_names, out_avals, zero_outs = [], [], [], []
    for alloc in nc.m.functions[0].allocations:
        if not isinstance(alloc, mybir.MemoryLocationSet):
            continue
        name = alloc.memorylocations[0].name
        if alloc.kind == "ExternalInput":
            if name != partition_name:
                in_names.append(name)
        elif alloc.kind == "ExternalOutput":
            out_names.append(name)
            shape = tuple(alloc.tensor_shape)
            dtype = mybir.dt.np(alloc.dtype)
            out_avals.append(jax.core.ShapedArray(shape, dtype))
            zero_outs.append(np.zeros(shape, dtype))
    all_names = in_names + out_names
    if partition_name is not None:
        all_names = all_names + [partition_name]
    all_names = tuple(all_names)
    n_in, n_out = len(in_names), len(out_names)

    def _body(*args):
        operands = list(args)
        if partition_name is not None:
            operands.append(partition_id_tensor())
        outs = _bass_exec_p.bind(
            *operands,
            out_avals=tuple(out_avals),
            in_names=all_names,
            out_names=tuple(out_names),
            lowering_input_output_aliases=(),
            sim_require_finite=True,
            sim_require_nnan=True,
            nc=nc,
        )
        return tuple(outs)

    devices = jax.devices()[:N_CORES]
    mesh = Mesh(np.asarray(devices), ("core",))
    sharded = jax.jit(
        shard_map(_body, mesh=mesh,
                  in_specs=(PartitionSpec("core"),) * (n_in + n_out),
                  out_specs=(PartitionSpec("core"),) * n_out,
                  check_rep=False),
        donate_argnums=tuple(range(n_in, n_in + n_out)),
        keep_unused=True,
    )
    _RUNNER = (sharded, in_names, out_names, zero_outs)
    return _RUNNER


def run_on_device(in_maps):
    sharded, in_names, out_names, zero_outs = get_runner()
    concat_in = [np.concatenate([m[n] for m in in_maps], axis=0)
                 for n in in_names]
    concat_zero = [np.zeros((N_CORES * z.shape[0], *z.shape[1:]), z.dtype)
                   for z in zero_outs]
    outs = sharded(*concat_in, *concat_zero)
    return {n: np.asarray(o) for n, o in zip(out_names, outs)}


def kernel(**inputs):
    in_maps = prep_in_maps(inputs)
    outs = run_on_device(in_maps)
    return outs["out"]  # per-core rows already concatenated in batch order


# revision 16
# speedup vs baseline: 182.1573x; 182.1573x over previous
"""GRU unit kernel for Trainium2, data-parallel over 8 NeuronCores.

Computation (per batch row):
    r  = sigmoid(x @ W_i2r + b_i2r + h @ W_h2r)
    z  = sigmoid(x @ W_i2z + b_i2z + h @ W_h2z)
    h1 = tanh   (x @ W_i2h + b_i2h + r * (h @ W_h2h))
    out = (1 - z) * h1 + z * h

Sharding: batch (16384) split 8 ways; weights replicated.

Host-side prep: x/h transposed to [K, B_local] and cast to bf16 (so the
stationary matmul operand loads directly, no on-chip transposes), the three
x-side / h-side weight matrices concatenated to [1024, 3072] bf16, biases
concatenated and pre-broadcast to [128, 3072] f32.

Device kernel per core (B_local=2048 rows = 16 m-tiles of 128):
  - weights + xT/hT resident in SBUF (bf16), h (f32) and out streamed.
  - per m-tile: 96 bf16 matmuls of N=512 into 8 PSUM banks
    (pre_r, pre_z, x@W_i2h, h@W_h2h, each split in two 512-halves),
    fp32 accumulation; epilogue on DVE (bias adds, blend) + ACT
    (sigmoid/tanh); result DMA'd out in fp32.
"""

import os
import numpy as np
import ml_dtypes
from contextlib import ExitStack

import concourse.bass as bass
import concourse.tile as tile
from concourse import bacc, mybir

# Walrus is invoked with --enable-ldw-opt=false by default; our inner loop
# issues runs of matmuls sharing one stationary operand, so redundant
# LDWEIGHTS dominate PE overhead. Flip the flag (verified bit-identical
# output vs the reference).
if os.environ.get("GRU_LDWOPT", "0") == "1":
    import concourse.bass_utils as _bu
    if not getattr(_bu, "_gru_ldwopt_patched", False):
        _orig_run_command = _bu.run_command

        def _run_command_ldwopt(argv, **kwargs):
            argv = ["--enable-ldw-opt=true" if a == "--enable-ldw-opt=false"
                    else a for a in argv]
            return _orig_run_command(argv, **kwargs)

        _bu.run_command = _run_command_ldwopt
        _bu._gru_ldwopt_patched = True

N_CORES = 8
B, I, H = 16384, 1024, 1024
BL = B // N_CORES           # 2048 batch rows per core
MT = BL // 128              # 16 m-tiles
KO = I // 128               # 8 k-tiles of 128
F32 = mybir.dt.float32
BF16 = mybir.dt.bfloat16
BF16_NP = ml_dtypes.bfloat16


def _ap_key(a):
    try:
        return (a.memref, a.offset, str(a.ap), str(a.dtype))
    except Exception:
        return ("?", id(a))


def dedupe_ldweights(nc):
    """Drop InstLdweights that reload the stationary tile already resident in
    the PE array (bacc emits one per matmul; walrus' ldw-opt can't be used on
    these). The paired InstMatmult keeps both APs, so data deps survive; the
    removed LDW's scheduling deps are merged into the following instruction."""
    total_removed = 0
    for blk in nc.m.functions[0].blocks:
        insts = list(blk.instructions)
        new = []
        last_key = None
        pending = []
        for i in insts:
            t = type(i).__name__
            eng = str(getattr(i, "engine", ""))
            if t == "InstLdweights":
                key = (_ap_key(i.ins[0]), str(i.perf_mode),
                       str(i.tile_position), str(i.is_transpose))
                if key == last_key:
                    pending.append(i)
                    total_removed += 1
                    continue
                last_key = key
                new.append(i)
            else:
                if "PE" in eng and t not in ("InstMatmult",
                                             "InstEventSemaphore"):
                    last_key = None  # unknown PE inst may clobber weights
                if pending and t == "InstMatmult":
                    for j in pending:
                        i.merge_dependencies_from(j)
                    pending = []
                new.append(i)
        if pending:
            # dangling dup LDWs at block end (shouldn't happen) — keep them
            new.extend(pending)
        blk.instructions = new
    return total_removed


def build_nc(reps: int = 1):
    nc = bacc.Bacc("TRN2", target_bir_lowering=False, debug=False,
                   num_devices=N_CORES)
    AF = mybir.ActivationFunctionType

    xt = nc.dram_tensor("xt", [I, BL], BF16, kind="ExternalInput").ap()
    ht = nc.dram_tensor("ht", [H, BL], BF16, kind="ExternalInput").ap()
    h32 = nc.dram_tensor("h32", [BL, H], F32, kind="ExternalInput").ap()
    wx = nc.dram_tensor("wx", [I, 3 * H], BF16, kind="ExternalInput").ap()
    wh = nc.dram_tensor("wh", [H, 3 * H], BF16, kind="ExternalInput").ap()
    bias = nc.dram_tensor("bias", [128, 3 * H], F32, kind="ExternalInput").ap()
    out = nc.dram_tensor("out", [BL, H], F32, kind="ExternalOutput").ap()

    with tile.TileContext(nc) as tc, ExitStack() as ctx:
        wpool = ctx.enter_context(tc.tile_pool(name="w", bufs=1))
        apool = ctx.enter_context(tc.tile_pool(name="a", bufs=1))
        hpool = ctx.enter_context(tc.tile_pool(name="h", bufs=2))
        epool = ctx.enter_context(tc.tile_pool(name="e", bufs=3))
        psum = ctx.enter_context(tc.tile_pool(name="ps", bufs=1, space="PSUM"))

        wx_sb = wpool.tile([128, KO, 3 * H], BF16, tag="wx")
        wh_sb = wpool.tile([128, KO, 3 * H], BF16, tag="wh")
        bias_sb = wpool.tile([128, 3 * H], F32, tag="bias")
        xt_sb = apool.tile([128, KO, BL], BF16, tag="xt")
        ht_sb = apool.tile([128, KO, BL], BF16, tag="ht")

        xt_r = xt.rearrange("(ko ki) b -> ki ko b", ki=128)
        ht_r = ht.rearrange("(ko ki) b -> ki ko b", ki=128)
        wx_r = wx.rearrange("(ko ki) n -> ki ko n", ki=128)
        wh_r = wh.rearrange("(ko ki) n -> ki ko n", ki=128)

        def body():
            emit_loads()
            for mt in range(MT):
                emit_mtile(mt)

        def emit_loads():
            # Load order sets DMA FIFO order on the sync ring. m-tile 0 runs
            # gate-major (r, z, a), so feed it: xt chunk0, wx[r], wx[z],
            # wx[a], then the h-side in the same pattern.
            CH = 4
            cw = BL // CH
            nc.sync.dma_start(xt_sb[:, :, 0:cw], xt_r[:, :, 0:cw])
            for g in range(3):
                nc.sync.dma_start(wx_sb[:, :, g * H:(g + 1) * H],
                                  wx_r[:, :, g * H:(g + 1) * H])
            nc.sync.dma_start(ht_sb[:, :, 0:cw], ht_r[:, :, 0:cw])
            for g in range(3):
                nc.sync.dma_start(wh_sb[:, :, g * H:(g + 1) * H],
                                  wh_r[:, :, g * H:(g + 1) * H])
            nc.sync.dma_start(bias_sb[:], bias)
            for c in range(1, CH):
                nc.sync.dma_start(xt_sb[:, :, c * cw:(c + 1) * cw],
                                  xt_r[:, :, c * cw:(c + 1) * cw])
                nc.sync.dma_start(ht_sb[:, :, c * cw:(c + 1) * cw],
                                  ht_r[:, :, c * cw:(c + 1) * cw])

        def emit_mtile(mt):
            ms = slice(mt * 128, (mt + 1) * 128)
            h32_t = hpool.tile([128, H], F32, tag="h32")
            nc.sync.dma_start(h32_t[:], h32[ms, :])

            # 8 PSUM banks: r/z get x-side + h-side accumulated; 'a' is
            # x@W_i2h (+bias later), 'b' is h@W_h2h (scaled by r later).
            ps = {}
            for g in ("r", "z", "a", "b"):
                for nh in range(2):
                    ps[(g, nh)] = psum.tile([128, 512], F32, tag=f"p{g}{nh}",
                                            name=f"p{g}{nh}")

            # x side: gates r,z,a read wx columns [0,H),[H,2H),[2H,3H).
            # Accumulation flags: r/z span x+h sides; a is x-only, b h-only.
            def mm_x(gi, g, ko, nh):
                o = nh * 512
                nc.tensor.matmul(ps[(g, nh)], xt_sb[:, ko, ms],
                                 wx_sb[:, ko, gi * H + o:gi * H + o + 512],
                                 start=(ko == 0),
                                 stop=(g == "a" and ko == KO - 1))

            def mm_h(gi, g, ko, nh):
                o = nh * 512
                nc.tensor.matmul(ps[(g, nh)], ht_sb[:, ko, ms],
                                 wh_sb[:, ko, gi * H + o:gi * H + o + 512],
                                 start=(g == "b" and ko == 0),
                                 stop=(ko == KO - 1))

            if mt == 0:
                # Gate-major: PE can start on the first wx gate chunk instead
                # of waiting for all of wx (costs extra LDWEIGHTS, only here).
                for gi, g in enumerate(("r", "z", "a")):
                    for ko in range(KO):
                        for nh in range(2):
                            mm_x(gi, g, ko, nh)
                for gi, g in enumerate(("r", "z", "b")):
                    for ko in range(KO):
                        for nh in range(2):
                            mm_h(gi, g, ko, nh)
            else:
                # ko-major: 6 consecutive matmuls share one stationary tile,
                # deduped to one LDWEIGHTS by walrus' ldw-opt.
                for ko in range(KO):
                    for nh in range(2):
                        for gi, g in enumerate(("r", "z", "a")):
                            mm_x(gi, g, ko, nh)
                for ko in range(KO):
                    for nh in range(2):
                        for gi, g in enumerate(("r", "z", "b")):
                            mm_h(gi, g, ko, nh)

            for nh in range(2):
                o = nh * 512
                nsl = slice(o, o + 512)
                pr, pz = ps[("r", nh)], ps[("z", nh)]
                pa, pb = ps[("a", nh)], ps[("b", nh)]
                tr = epool.tile([128, 512], F32, tag="tr")
                tz = epool.tile([128, 512], F32, tag="tz")
                ta = epool.tile([128, 512], F32, tag="ta")
                nc.vector.tensor_add(tr[:], pr[:], bias_sb[:, o:o + 512])
                nc.scalar.activation(tr[:], tr[:], AF.Sigmoid)       # r
                nc.vector.tensor_add(tz[:], pz[:], bias_sb[:, H + o:H + o + 512])
                nc.scalar.activation(tz[:], tz[:], AF.Sigmoid)       # z
                nc.vector.tensor_add(ta[:], pa[:],
                                     bias_sb[:, 2 * H + o:2 * H + o + 512])
                nc.vector.tensor_mul(tr[:], tr[:], pb[:])            # r*(hU)
                nc.vector.tensor_add(ta[:], ta[:], tr[:])
                nc.scalar.activation(ta[:], ta[:], AF.Tanh)          # h1
                nc.vector.tensor_sub(tr[:], h32_t[:, nsl], ta[:])    # h-h1
                nc.vector.tensor_mul(tr[:], tz[:], tr[:])            # z*(h-h1)
                nc.vector.tensor_add(tr[:], ta[:], tr[:])            # out
                nc.scalar.dma_start(out[ms, nsl], tr[:])

        if reps > 1:
            with tc.For_i(0, reps, 1):
                body()
        else:
            body()

    nc.compile()
    if os.environ.get("GRU_DEDUP", "1") == "1":
        dedupe_ldweights(nc)
    return nc


def prep_in_maps(inputs):
    """Host-side marshalling: shard batch, transpose+cast activations,
    concat weights/biases. Returns per-core input dicts."""
    g = {k: np.asarray(v) for k, v in inputs.items()}
    x, h = g["inputs"], g["hidden"]
    wx = np.concatenate([g["W_i2r"], g["W_i2z"], g["W_i2h"]], axis=1)
    wh = np.concatenate([g["W_h2r"], g["W_h2z"], g["W_h2h"]], axis=1)
    wx = np.ascontiguousarray(wx).astype(BF16_NP)
    wh = np.ascontiguousarray(wh).astype(BF16_NP)
    b = np.concatenate([g["b_i2r"], g["b_i2z"], g["b_i2h"]]).astype(np.float32)
    bias_b = np.ascontiguousarray(np.broadcast_to(b, (128, 3 * H)))
    xt_all = np.ascontiguousarray(x.T).astype(BF16_NP)
    ht_all = np.ascontiguousarray(h.T).astype(BF16_NP)
    in_maps = []
    for c in range(N_CORES):
        sl = slice(c * BL, (c + 1) * BL)
        in_maps.append({
            "xt": np.ascontiguousarray(xt_all[:, sl]),
            "ht": np.ascontiguousarray(ht_all[:, sl]),
            "h32": np.ascontiguousarray(h[sl].astype(np.float32)),
            "wx": wx,
            "wh": wh,
            "bias": bias_b,
        })
    return in_maps


_RUNNERS = {}


def get_runner(reps: int = 1):
    """Build the bass module once and wrap it in a jitted 8-way shard_map,
    mirroring concourse.bass2jax.run_bass_via_pjrt but reusable across calls
    (so repeated executions don't re-trace/re-compile). reps>1 wraps the
    whole kernel in an on-device loop (for timing via amortization)."""
    if reps in _RUNNERS:
        return _RUNNERS[reps]
    import jax
    from jax.sharding import Mesh, PartitionSpec
    from jax.experimental.shard_map import shard_map
    from concourse.bass2jax import (_bass_exec_p, install_neuronx_cc_hook,
                                    partition_id_tensor)

    nc = build_nc(reps)
    install_neuronx_cc_hook()

    partition_name = (nc.partition_id_tensor.name
                      if nc.partition_id_tensor else None)
    in_names, out_names, out_avals, zero_outs = [], [], [], []
    for alloc in nc.m.functions[0].allocations:
        if not isinstance(alloc, mybir.MemoryLocationSet):
            continue
        name = alloc.memorylocations[0].name
        if alloc.kind == "ExternalInput":
            if name != partition_name:
                in_names.append(name)
        elif alloc.kind == "ExternalOutput":
            out_names.append(name)
            shape = tuple(alloc.tensor_shape)
            dtype = mybir.dt.np(alloc.dtype)
            out_avals.append(jax.core.ShapedArray(shape, dtype))
            zero_outs.append(np.zeros(shape, dtype))
    all_names = in_names + out_names
    if partition_name is not None:
        all_names = all_names + [partition_name]
    all_names = tuple(all_names)
    n_in, n_out = len(in_names), len(out_names)

    def _body(*args):
        operands = list(args)
        if partition_name is not None:
            operands.append(partition_id_tensor())
        outs = _bass_exec_p.bind(
            *operands,
            out_avals=tuple(out_avals),
            in_names=all_names,
            out_names=tuple(out_names),
            lowering_input_output_aliases=(),
            sim_require_finite=True,
            sim_require_nnan=True,
            nc=nc,
        )
        return tuple(outs)

    devices = jax.devices()[:N_CORES]
    mesh = Mesh(np.asarray(devices), ("core",))
    sharded = jax.jit(
        shard_map(_body, mesh=mesh,
                  in_specs=(PartitionSpec("core"),) * (n_in + n_out),
                  out_specs=(PartitionSpec("core"),) * n_out,
                  check_rep=False),
        donate_argnums=tuple(range(n_in, n_in + n_out)),
        keep_unused=True,
    )
    _RUNNERS[reps] = (sharded, in_names, out_names, zero_outs)
    return _RUNNERS[reps]


def run_on_device(in_maps):
    sharded, in_names, out_names, zero_outs = get_runner()
    concat_in = [np.concatenate([m[n] for m in in_maps], axis=0)
                 for n in in_names]
    concat_zero = [np.zeros((N_CORES * z.shape[0], *z.shape[1:]), z.dtype)
                   for z in zero_outs]
    outs = sharded(*concat_in, *concat_zero)
    return {n: np.asarray(o) for n, o in zip(out_names, outs)}


def kernel(**inputs):
    in_maps = prep_in_maps(inputs)
    outs = run_on_device(in_maps)
    return outs["out"]  # per-core rows already concatenated in batch order
